# revision 1
# baseline (speedup 1.0000x reference)
"""EntitySelector sparse-attention kernel for 8 Trainium2 NeuronCores.

Sharding: data-parallel over batch (16 batches -> 2 per core), no collectives.

Algorithm (algebraically restructured vs the naive reference):
  K  = gather(ent_emb) @ WK^T + bk              (NB=256 rows per batch)
  EQ = K @ WQ          -> scores = query @ EQ^T (query is never projected)
  EO = K @ WO^T + sqrt(d)*bo                    (WO + bias folded onto entities)
  scores += (bq . K^T) - 30000*(1-mask)         (bias+mask row via a K=1 matmul)
  probs = exp(scores - rowmax)  [accum_out gives rowsum for free]
  out_pre = probs @ EO          [1/rowsum cancels into the LayerNorm scale]
  LN: out = (out_pre - mu) * Kln,  Kln = 1/sqrt(var + eps*d*rowsum^2)

This moves both D x D projections from the L=1024 query rows onto the 256
entity rows (2.2x less PE work) and collapses softmax/LN elementwise work.
"""

import sys

sys.path.insert(0, "/opt/trn_rl_repo")

import numpy as np
import ml_dtypes

import concourse.bass as bass
import concourse.mybir as mybir
import concourse.tile as tile
from concourse.tile_rust import add_dep_helper
from concourse import bacc
from concourse.bass_utils import run_bass_kernel_spmd
from concourse.masks import make_identity

P = 128
D = 1024
DT = D // P            # 8 feature tiles
BL = 2                 # batches per core
L = 1024
LC = 512               # l-chunk
NLC = L // LC          # 2 chunks
LT = LC // P           # 4 l-tiles per chunk
NB = 256
NT = NB // P           # 2 entity tiles
NE = 20000
NCORES = 8

F32 = mybir.dt.float32
F32R = mybir.dt.float32r
BF16 = mybir.dt.bfloat16
I32 = mybir.dt.int32

AF = mybir.ActivationFunctionType
OP = mybir.AluOpType
AX = mybir.AxisListType

EPS_SD = float(np.sqrt(1e-5 * D))   # Square(EPS_SD*rsum) = eps*d*rsum^2
SQRT_D = float(np.sqrt(D))

# BIR requires matmul operand dtypes to match when f32/f32r is involved, so
# the whole entity-transform path (weights, gathered entities, K) runs in one
# dtype: bf16 (fast DMA, ~1e-2 rel err) or f32r (safe, more weight DMA).
WEIGHTS_BF16 = True

_CACHE = {}


class _Ctx:
    pass


def _emit_gather(nc, g, b):
    """Indirect-gather this batch's entities; returns gather instructions."""
    idx_col = g.idxp.tile([P, NT], I32, tag="idxc")
    nc.scalar.dma_start(idx_col, g.idx[b].rearrange("(t p) -> p t", p=P))
    mrow = g.rowp.tile([1, NB], F32, tag="mrow")
    nc.scalar.dma_start(mrow, g.msk[b:b + 1, :])
    g.mrow[b] = mrow
    tiles, insts = [], []
    for nt in range(NT):
        e = g.gp.tile([P, D], F32R, tag="g")
        gi = nc.gpsimd.indirect_dma_start(
            out=e[:], out_offset=None, in_=g.emb[:, :],
            in_offset=bass.IndirectOffsetOnAxis(ap=idx_col[:, nt:nt + 1], axis=0))
        tiles.append(e)
        insts.append(gi)
    g.gtiles[b] = tiles
    return insts


def _emit_entT(nc, g, b):
    """Feature-major transpose of the gathered entities (2 dt per psum)."""
    entT = g.entp.tile([P, DT, NB], g.wdt, tag="entT", name=f"entT{b}")
    pts = [g.ps_big.tile([P, LC], F32R, tag="big", name=f"pt{b}{h}")
           for h in range(DT // 2)]
    lasts = []
    # all transposes of the first gathered tile run before any of the
    # second, so PE is not blocked on the later gather transfer
    for nt in range(NT):
        for h in range(DT // 2):
            for j in range(2):
                dt = 2 * h + j
                last = nc.tensor.transpose(
                    pts[h][:, j * NB + nt * P:j * NB + (nt + 1) * P],
                    g.gtiles[b][nt][:, dt * P:(dt + 1) * P], g.ident_r)
            if nt == NT - 1:
                lasts.append(last)
    for h in range(DT // 2):
        nc.vector.tensor_copy(entT[:, 2 * h:2 * h + 2, :],
                              pts[h].rearrange("p (a n) -> p a n", a=2))
    g.entT[b] = entT
    return lasts


def _emit_K(nc, g, b, dos):
    """K^T feature-major: kt[do, n] = (G @ WK^T + bk)^T."""
    if b not in g.kt:
        g.kt[b] = g.ktp.tile([P, DT, NB], g.wdt, tag="kt", name=f"kt{b}")
    kt, entT = g.kt[b], g.entT[b]
    lasts = []
    for dp in range(dos.start // 2, dos.stop // 2):
        p = g.ps_big.tile([P, LC], F32, tag="big")
        for j in range(2):
            do = 2 * dp + j
            pj = p[:, j * NB:(j + 1) * NB]
            for k in range(DT):
                nc.tensor.matmul(pj, g.wk_sb[:, k, do * P:(do + 1) * P],
                                 entT[:, k, :], start=(k == 0), stop=False)
            i = nc.tensor.matmul(pj, g.bk_row[:, do * P:(do + 1) * P],
                                 g.ones256, start=False, stop=True)
            lasts.append(i)
        nc.scalar.copy(kt[:, 2 * dp:2 * dp + 2, :],
                       p.rearrange("p (a n) -> p a n", a=2))
    return lasts


def _emit_EQ(nc, g, b, prs):
    """EQ^T feature-major: eqt[e, n] = (K @ WQ)^T. prs = et-pair indices."""
    if b not in g.eqt:
        g.eqt[b] = g.eqp.tile([P, DT, NB], F32R, tag="eqt", name=f"eqt{b}")
    eqt, kt = g.eqt[b], g.kt[b]
    lasts = []
    for pr in prs:
        p = g.ps_big.tile([P, LC], F32, tag="big")
        for j in range(2):
            et = 2 * pr + j
            for k in range(DT):
                i = nc.tensor.matmul(p[:, j * NB:(j + 1) * NB],
                                     g.wq_sb[:, k, et * P:(et + 1) * P],
                                     kt[:, k, :], start=(k == 0),
                                     stop=(k == DT - 1))
        lasts.append(i)
        nc.scalar.copy(eqt[:, 2 * pr:2 * pr + 2, :],
                       p.rearrange("p (a n) -> p a n", a=2))
    return lasts


def _emit_crow(nc, g, b):
    """logm2 row: bq . K^T  - 30000*(1-mask), [1, NB] f32r."""
    kt = g.kt[b]
    cfull = g.ps_sc.tile([P, NB], F32, tag="sc")
    cps = cfull[0:1, :]
    for k in range(DT):
        nc.tensor.matmul(cps, g.bq_col[:, k:k + 1], kt[:, k, :],
                         start=(k == 0), stop=(k == DT - 1))
    trow = g.rowp.tile([1, NB], F32, tag="trow")
    nc.vector.tensor_scalar(out=trow, in0=g.mrow[b], scalar1=-1.0,
                            scalar2=30000.0, op0=OP.add, op1=OP.mult)
    lg = g.rowp.tile([1, NB], F32R, tag="lg")
    nc.vector.tensor_add(lg, cps, trow)
    g.lg[b] = lg


def _emit_EO_half(nc, g, b, half):
    """EO entity-major for one output half (both nt blocks).

    eo[n, e] = K @ WO^T + sqrt(d)*bo (bias pre-scaled on host)."""
    if b not in g.eo:
        g.eo[b] = g.eop.tile([P, NT, D], BF16, tag="eo", name=f"eo{b}")
    kt = g.kt[b]
    for nt in range(NT):
        p = g.ps_big.tile([P, LC], F32, tag="big")
        for k in range(DT):
            nc.tensor.matmul(p, kt[:, k, nt * P:(nt + 1) * P],
                             g.wo_sb[:, k, half * LC:(half + 1) * LC],
                             start=(k == 0), stop=False)
        nc.tensor.matmul(p, g.ones256[:, 0:P],
                         g.bo_row[:, half * LC:(half + 1) * LC],
                         start=False, stop=True)
        nc.scalar.copy(g.eo[b][:, nt, half * LC:(half + 1) * LC], p)


def _load_qin(nc, g, b, lc):
    """Transposed query chunk [P, DT, LC] on the sync queue (2 DMAs)."""
    qin = g.qp.tile([P, DT, LC], F32R, tag="qin", name=f"qin{b}{lc}")
    qTb = g.qT[b].rearrange("(kt p) l -> p kt l", p=P)
    h = DT // 2
    i = nc.sync.dma_start(qin[:, :h, :], qTb[:, :h, lc * LC:(lc + 1) * LC])
    j = nc.sync.dma_start(qin[:, h:, :], qTb[:, h:, lc * LC:(lc + 1) * LC])
    g.qins[(b, lc)] = qin
    return i, j


def _tile_idx(ti):
    return ti // 8, (ti // 4) % 2, ti % 4          # (batch, chunk, tile)


def _emit_psc(nc, g, ti):
    """Scores + start of softmax for global l-tile ti."""
    b, lc, t = _tile_idx(ti)
    qin, eqt, lg = g.qins[(b, lc)], g.eqt[b], g.lg[b]
    psc = g.ps_sc.tile([P, NB], F32, tag="sc")
    for et in range(DT):
        nc.tensor.matmul(psc, qin[:, et, t * P:(t + 1) * P], eqt[:, et, :],
                         start=(et == 0), stop=False)
    g.psc_last[ti] = nc.tensor.matmul(psc, g.onesr[:, 0:P], lg,
                                      start=False, stop=True)
    negmax = g.lnp.tile([P, 1], F32, tag="nm")
    nc.vector.reduce_max(negmax, psc, axis=AX.X, negate=True)
    probs = g.probsp.tile([P, NB], BF16, tag="pr")
    rsum = g.lnp.tile([P, 1], F32, tag="rs")
    nc.scalar.activation(out=probs, in_=psc, func=AF.Exp, bias=negmax,
                         scale=1.0, accum_out=rsum)
    return probs, rsum


def _emit_finA(nc, g, ti, b, probs, rsum):
    """Transpose probs, value matmuls, LN stats."""
    ptb = g.ps_trb.tile([P, NB], BF16, tag="trb")
    for nt in range(NT):
        nc.tensor.transpose(ptb[:, nt * P:(nt + 1) * P],
                            probs[:, nt * P:(nt + 1) * P], g.ident_b)
    pT = g.ptp.tile([P, NT, P], BF16, tag="pT")
    nc.scalar.copy(pT, ptb.rearrange("p (a c) -> p a c", a=NT))
    sq = g.lnp.tile([P, 1], F32, tag="sq")
    nc.scalar.activation(out=sq, in_=rsum, func=AF.Square, scale=EPS_SD)
    eo = g.eo[b]
    stats = g.lnp.tile([P, 2, 6], F32, tag="st")
    po = []
    for half in range(2):
        p = g.ps_big.tile([P, LC], F32, tag="big")
        for nt in range(NT):
            nc.tensor.matmul(p, pT[:, nt, :],
                             eo[:, nt, half * LC:(half + 1) * LC],
                             start=(nt == 0), stop=(nt == NT - 1))
        nc.vector.bn_stats(out=stats[:, half, :], in_=p)
        po.append(p)
    mv = g.lnp.tile([P, 2], F32, tag="mv")
    nc.vector.bn_aggr(out=mv, in_=stats)
    return po, mv, sq


def _emit_finB(nc, g, b, lc, t, po, mv, sq):
    """LN scale + fused normalize + store."""
    ti = (b * NLC + lc) * LT + t
    lt = g.lnp.tile([P, 1], F32, tag="lt")
    nc.scalar.activation(out=lt, in_=mv[:, 1:2], func=AF.Ln, bias=sq,
                         scale=1.0)
    kln = g.lnp.tile([P, 1], F32, tag="kl")
    nc.scalar.activation(out=kln, in_=lt, func=AF.Exp, scale=-0.5)
    o = g.opool.tile([P, D], F32, tag="o")
    m2 = g.lnp.tile([P, 1], F32, tag="m2")
    nc.gpsimd.tensor_scalar(out=m2, in0=mv[:, 0:1], scalar1=kln,
                            scalar2=-1.0, op0=OP.mult, op1=OP.mult)
    nc.scalar.activation(out=o[:, 0:LC], in_=po[0], func=AF.Identity,
                         bias=m2, scale=kln)
    r0 = (lc * LT + t) * P
    split = ti >= 12
    if split:
        nc.sync.dma_start(g.out[b, r0:r0 + P, 0:LC], o[:, 0:LC])
    nc.vector.tensor_scalar(out=o[:, LC:D], in0=po[1], scalar1=mv[:, 0:1],
                            scalar2=kln, op0=OP.subtract, op1=OP.mult)
    if g.apply_affine:
        nc.vector.tensor_mul(o, o, g.lng_bc)
        nc.vector.tensor_add(o, o, g.lnb_bc)
    if split:
        nc.sync.dma_start(g.out[b, r0:r0 + P, LC:D], o[:, LC:D])
    else:
        nc.sync.dma_start(g.out[b, r0:r0 + P, :], o)


def _emit_pipeline(nc, g, fills):
    """Global software pipeline over all 16 l-tiles with PE filler slots."""
    NTI = BL * NLC * LT
    st = {}

    def psc(ti):
        st[ti] = _emit_psc(nc, g, ti)

    def fill(i):
        for f in fills.get(i, []):
            f()

    # prologue: prime 4 score tiles, then EO(b0)/entT(b1) PE fillers
    for ti in range(2):
        psc(ti)
        g.hook(ti)
    fill("pro")
    for i in range(NTI):
        b, lc, t = _tile_idx(i)
        st[i] = _emit_finA(nc, g, i, b, *st[i])
        if i >= 1:
            bb, plc, pt = _tile_idx(i - 1)
            _emit_finB(nc, g, bb, plc, pt, *st.pop(i - 1))
        fill(i)
        if i + 2 < NTI:
            psc(i + 2)
    bb, plc, pt = _tile_idx(NTI - 1)
    _emit_finB(nc, g, bb, plc, pt, *st.pop(NTI - 1))


def build_nc(apply_affine):
    nc = bacc.Bacc("TRN2", target_bir_lowering=False, debug=False,
                   num_devices=NCORES)
    g = _Ctx()
    g.apply_affine = apply_affine
    g.wdt = BF16 if WEIGHTS_BF16 else F32R
    g.gtiles, g.mrow, g.lg, g.qins = {}, {}, {}, {}
    g.psc_last = {}
    g.entT, g.kt, g.eqt, g.eo = {}, {}, {}, {}

    g.qT = nc.dram_tensor("qT", [BL, D, L], F32R, kind="ExternalInput")
    g.emb = nc.dram_tensor("emb", [NE, D], F32R, kind="ExternalInput")
    g.idx = nc.dram_tensor("idx", [BL, NB], I32, kind="ExternalInput")
    g.msk = nc.dram_tensor("msk", [BL, NB], F32, kind="ExternalInput")
    WDT = BF16 if WEIGHTS_BF16 else F32R
    wk = nc.dram_tensor("wk", [D, D], WDT, kind="ExternalInput")
    wq = nc.dram_tensor("wq", [D, D], WDT, kind="ExternalInput")
    wo = nc.dram_tensor("wo", [D, D], WDT, kind="ExternalInput")
    bq = nc.dram_tensor("bq", [D], WDT, kind="ExternalInput")
    bk = nc.dram_tensor("bk", [D], WDT, kind="ExternalInput")
    bo = nc.dram_tensor("bo", [D], WDT, kind="ExternalInput")
    ones_w = nc.dram_tensor("ones_w", [NB], WDT, kind="ExternalInput")
    ones_r = nc.dram_tensor("ones_r", [NB], F32R, kind="ExternalInput")
    if apply_affine:
        lng = nc.dram_tensor("lng", [D], F32, kind="ExternalInput")
        lnb = nc.dram_tensor("lnb", [D], F32, kind="ExternalInput")
    g.out = nc.dram_tensor("out", [BL, L, D], F32, kind="ExternalOutput")

    def bcast_row(dram_1d):
        ap = dram_1d[:]
        return bass.AP(tensor=ap.tensor, offset=ap.offset,
                       ap=[[0, P]] + list(ap.ap))

    with tile.TileContext(nc) as tc:
        with (
            tc.tile_pool(name="wpool", bufs=1) as wpool,
            tc.tile_pool(name="gp", bufs=2) as gp,
            tc.tile_pool(name="idxp", bufs=2) as idxp,
            tc.tile_pool(name="rowp", bufs=1) as rowp,
            tc.tile_pool(name="entp", bufs=1) as entp,
            tc.tile_pool(name="ktp", bufs=2) as ktp,
            tc.tile_pool(name="eqp", bufs=2) as eqp,
            tc.tile_pool(name="eop", bufs=2) as eop,
            tc.tile_pool(name="qp", bufs=2) as qp,
            tc.tile_pool(name="probsp", bufs=7) as probsp,
            tc.tile_pool(name="ptp", bufs=4) as ptp,
            tc.tile_pool(name="lnp", bufs=7) as lnp,
            tc.tile_pool(name="op", bufs=3) as opool,
            tc.tile_pool(name="ps_sc", bufs=3, space="PSUM") as ps_sc,
            tc.tile_pool(name="ps_big", bufs=4, space="PSUM") as ps_big,
            tc.tile_pool(name="ps_trb", bufs=1, space="PSUM") as ps_trb,
        ):
            g.gp, g.idxp, g.rowp, g.entp = gp, idxp, rowp, entp
            g.ktp, g.eqp, g.eop, g.qp = ktp, eqp, eop, qp
            g.probsp, g.ptp, g.lnp, g.opool = probsp, ptp, lnp, opool
            g.ps_sc, g.ps_big, g.ps_trb = ps_sc, ps_big, ps_trb

            nc.scalar.add_instruction(mybir.InstLoadActFuncSet(
                name=nc.get_next_instruction_name(), act_func_set_id=6,
                ins=[], outs=[]))
            ident = wpool.tile([P, P], F32)
            make_identity(nc, ident)
            g.ident_r = wpool.tile([P, P], F32R)
            nc.vector.tensor_copy(g.ident_r, ident)
            g.ident_b = wpool.tile([P, P], BF16)
            nc.vector.tensor_copy(g.ident_b, ident)
            # keep the PE p-state ramp warm while the gathers land: the
            # cost model halves matmul speed after any idle gap until 3us
            # of continuous execution
            for _ in range(28):
                w = g.ps_trb.tile([P, P], BF16, tag="trb")
                nc.tensor.transpose(w, g.ident_b, g.ident_b)

            # gathers first on the SWDGE ring (idx rows ride fast HWDGE)
            gins0 = _emit_gather(nc, g, 0)
            gins1 = _emit_gather(nc, g, 1)

            # tiny bias rows early on HWDGE (negligible FIFO footprint)
            g.bk_row = wpool.tile([1, D], WDT)
            nc.scalar.dma_start(g.bk_row, bk[:].rearrange("(a d) -> a d", a=1))
            g.bq_col = wpool.tile([P, DT], WDT)
            nc.scalar.dma_start(g.bq_col, bq[:].rearrange("(t p) -> p t", p=P))
            g.bo_row = wpool.tile([1, D], WDT)
            nc.scalar.dma_start(g.bo_row, bo[:].rearrange("(a d) -> a d", a=1))
            g.ones256 = wpool.tile([1, NB], WDT)
            nc.scalar.dma_start(g.ones256,
                                ones_w[:].rearrange("(a d) -> a d", a=1))
            g.onesr = wpool.tile([1, NB], F32R)
            nc.scalar.dma_start(g.onesr,
                                ones_r[:].rearrange("(a d) -> a d", a=1))

            # batch-0 transform; each weight-stage DMA is "pulled" into the
            # DMA-engine FIFO by a dep on compute whose dispatch is gated on
            # the previous stage's data
            ent_lasts = _emit_entT(nc, g, 0)

            CW = 2 * P if WEIGHTS_BF16 else P
            g.wk_sb = wpool.tile([P, DT, D], WDT)
            wk_r = wk[:, :].rearrange("(kt p) m -> p kt m", p=P)
            if WEIGHTS_BF16:
                wk_anchors = [gins0[0], gins0[1], ent_lasts[1], ent_lasts[2]]
            else:
                wk_anchors = [gins0[0], gins0[0], gins0[1], gins0[1],
                              ent_lasts[1], ent_lasts[1],
                              ent_lasts[2], ent_lasts[2]]
            for c in range(D // CW):
                i = nc.scalar.dma_start(g.wk_sb[:, :, c * CW:(c + 1) * CW],
                                        wk_r[:, :, c * CW:(c + 1) * CW])
                add_dep_helper(i.ins, wk_anchors[c].ins,
                               reason="wk chunk staggered behind gathers")

            k_lasts = _emit_K(nc, g, 0, range(DT))
            g.wq_sb = wpool.tile([P, DT, D], WDT)
            wq_r = wq[:, :].rearrange("(kt p) m -> p kt m", p=P)
            for c in range(D // CW):
                i = nc.sync.dma_start(g.wq_sb[:, :, c * CW:(c + 1) * CW],
                                      wq_r[:, :, c * CW:(c + 1) * CW])
                add_dep_helper(i.ins, k_lasts[2 * c if WEIGHTS_BF16 else c].ins,
                               reason="wq chunk staggered behind wk")

            eq_lasts = _emit_EQ(nc, g, 0, range(DT // 2))
            q00a, q00b = _load_qin(nc, g, 0, 0)
            add_dep_helper(q00a.ins, eq_lasts[0].ins, reason="qin after wq")
            add_dep_helper(q00b.ins, eq_lasts[1].ins, reason="qin after wq")
            _emit_crow(nc, g, 0)

            # wo/bo/qin01 are emitted inside the pipeline, anchored on the
            # first score tiles so they trail the critical-path DMAs
            g.wo_sb = wpool.tile([P, DT, D], WDT)
            g.bo_bc = wpool.tile([P, D], F32)
            wo_r = wo[:, :].rearrange("(kt p) m -> p kt m", p=P)

            nwo = 2 if WEIGHTS_BF16 else 4
            wcw = D // nwo
            for c in range(nwo):
                i = nc.sync.dma_start(g.wo_sb[:, :, c * wcw:(c + 1) * wcw],
                                      wo_r[:, :, c * wcw:(c + 1) * wcw])
                add_dep_helper(i.ins, eq_lasts[2 + c // (nwo // 2)].ins,
                               reason="wo behind qin chunk0")
            if apply_affine:
                i = nc.sync.dma_start(g.lng_bc, bcast_row(lng))
                add_dep_helper(i.ins, eq_lasts[3].ins, reason="ln")
                i = nc.sync.dma_start(g.lnb_bc, bcast_row(lnb))
                add_dep_helper(i.ins, eq_lasts[3].ins, reason="ln")

            def hook(ti):
                if ti < 2:
                    add_dep_helper(gins1[ti].ins, g.psc_last[ti].ins,
                                   reason="b1 gathers behind wo")
                if ti == 1:
                    a_, b_ = _load_qin(nc, g, 0, 1)
                    add_dep_helper(a_.ins, g.psc_last[1].ins, reason="qin01")
                    add_dep_helper(b_.ins, g.psc_last[1].ins, reason="qin01")

            g.hook = hook
            if apply_affine:
                g.lng_bc = wpool.tile([P, D], F32)
                g.lnb_bc = wpool.tile([P, D], F32)

            def f_pro():
                _emit_EO_half(nc, g, 0, 0)
                _emit_EO_half(nc, g, 0, 1)

            def f_qin(b, lc, anchor_ti):
                a_, b_ = _load_qin(nc, g, b, lc)
                add_dep_helper(a_.ins, g.psc_last[anchor_ti].ins,
                               reason="qin staggered")
                add_dep_helper(b_.ins, g.psc_last[anchor_ti].ins,
                               reason="qin staggered")

            fills = {
                "pro": [f_pro],
                2: [lambda: _emit_entT(nc, g, 1)],
                3: [lambda: _emit_K(nc, g, 1, range(0, 4)),
                    lambda: f_qin(1, 0, 4)],
                4: [lambda: _emit_K(nc, g, 1, range(4, 8))],
                5: [lambda: _emit_EQ(nc, g, 1, range(0, 2))],
                6: [lambda: _emit_EQ(nc, g, 1, range(2, 4)),
                    lambda: _emit_crow(nc, g, 1)],
                7: [lambda: _emit_EO_half(nc, g, 1, 0),
                    lambda: _emit_EO_half(nc, g, 1, 1),
                    lambda: f_qin(1, 1, 8)],
            }
            _emit_pipeline(nc, g, fills)

    nc.compile()
    return nc


def _get_nc(apply_affine):
    key = bool(apply_affine)
    if key not in _CACHE:
        _CACHE[key] = build_nc(key)
    return _CACHE[key]


def kernel(query, ent_emb, ent_idx_in_batch, max_entity_number,
           WQ_w, WQ_b, WK_w, WK_b, WO_w, WO_b, ln_g, ln_b):
    query = np.asarray(query, np.float32)
    ent_emb = np.ascontiguousarray(np.asarray(ent_emb, np.float32))
    idx = np.asarray(ent_idx_in_batch)
    mask = (idx != -1).astype(np.float32)
    idx32 = np.where(idx < 0, 0, idx).astype(np.int32)
    wdt = ml_dtypes.bfloat16 if WEIGHTS_BF16 else np.float32
    wq = np.ascontiguousarray(np.asarray(WQ_w, np.float32)).astype(wdt)
    wk = np.ascontiguousarray(np.asarray(WK_w, np.float32).T).astype(wdt)
    wo = np.ascontiguousarray(np.asarray(WO_w, np.float32).T).astype(wdt)
    bq = np.ascontiguousarray(np.asarray(WQ_b, np.float32)).astype(wdt)
    bk = np.ascontiguousarray(np.asarray(WK_b, np.float32)).astype(wdt)
    bo = np.ascontiguousarray(
        np.asarray(WO_b, np.float32) * SQRT_D).astype(wdt)
    lng = np.asarray(ln_g, np.float32)
    lnb = np.asarray(ln_b, np.float32)
    apply_affine = not (np.all(lng == 1.0) and np.all(lnb == 0.0))

    qT = np.ascontiguousarray(query.transpose(0, 2, 1))  # (B, D, L)

    nc = _get_nc(apply_affine)
    in_maps = []
    for c in range(NCORES):
        s = slice(c * BL, (c + 1) * BL)
        m = dict(
            qT=np.ascontiguousarray(qT[s]),
            emb=ent_emb,
            idx=np.ascontiguousarray(idx32[s]),
            msk=np.ascontiguousarray(mask[s]),
            wq=wq, wk=wk, wo=wo, bq=bq, bk=bk, bo=bo,
            ones_w=np.ones(NB, np.float32).astype(wdt),
            ones_r=np.ones(NB, np.float32),
        )
        if apply_affine:
            m["lng"] = lng
            m["lnb"] = lnb
        in_maps.append(m)

    res = run_bass_kernel_spmd(nc, in_maps, core_ids=list(range(NCORES)))
    return np.concatenate([r["out"] for r in res.results], axis=0)

